# revision 22
# baseline (speedup 1.0000x reference)
"""Trainium2 Bass kernel for a SwiGLU-style feed-forward block.

reference:
    gate = x @ w1.T ; up = x @ w2.T ; h = silu(gate) * up ; out = h @ w3.T
    x: [4, 2048, 2048] f32, w1/w2: [8192, 2048] f32, w3: [2048, 8192] f32

Strategy: pure data-parallel over the 8192 tokens - each of the 8
NeuronCores gets 1024 tokens and the full weights, processed in two
512-token chunks.  The PE runs at ~99% occupancy in bf16, so the
remaining speedup comes from moving a slice of each contraction to
fp8e4m3 DoubleRow matmuls (2x rate): the first UP_PAIRS*256 contraction
elements of the up-projection and the first DOWN_PAIRS*256 elements of
the down-projection run as fp8 DoubleRow, the rest stays bf16.  Both
parts accumulate into the same PSUM bank: the fp8 operand scales
multiply to 8192 = 2^13, and the bf16 weights are pre-scaled by 8192 on
the host (exact, power of two), so every contribution carries the same
scale which is divided out on the PSUM->SBUF path (ACT silu scale /
DVE scalar-mul).  Error budget (measured vs reference): bf16 0.0044,
mixed config (up 1 pair, down 3 pairs) 0.018 < 0.02 gate.

All DRAM tensors are host-pre-arranged so every DMA unit (x chunk,
w1/w2 slab, w3 e-tile) is contiguous per partition (4-32KB runs),
keeping descriptor counts ~30x below the naive strided layout.
"""

import json

import numpy as np
import ml_dtypes

import concourse.bass as bass
import concourse.mybir as mybir
import concourse.tile as tile
from concourse.vector_clock import ScopedClock
from concourse.bass_utils import run_bass_kernel_spmd

# ---------------------------------------------------------------- shapes
N_CORES = 8
EMB = 2048          # E
HID = 8192          # H
T_TOTAL = 8192      # B*S tokens
T_SHARD = T_TOTAL // N_CORES   # 1024 tokens per core
T_CHUNK = 512                  # tokens per on-chip pass
N_CHUNKS = T_SHARD // T_CHUNK
E_SUB = EMB // 128             # 16 contraction subtiles for phase A
H_SUB = HID // 128             # 64 contraction subtiles for phase B
HT_TOTAL = HID // 128          # 64 h-tiles

# fp8 mixing config: number of 256-deep contraction pairs done in fp8
GP = 0                         # gate pairs (phase A)
UP = 1                         # up pairs (phase A)
DP = 4                         # down pairs (phase B)
AP8 = max(GP, UP)              # x fp8 slices stored = 2*AP8

# scales: fp8 operand scales multiply to PSUM_SCALE, bf16 weights carry
# PSUM_SCALE directly (exact power-of-two shifts).
SX8 = 16.0                     # x -> fp8
SW12 = 512.0                   # w1/w2 -> fp8
SH8 = 4.0                      # h -> fp8
SW3 = 2048.0                   # w3 -> fp8
PSUM_SCALE = 8192.0            # = SX8*SW12 = SH8*SW3
INV_PSUM = 1.0 / PSUM_SCALE    # 2^-13
HT_TO_H8 = SH8 / PSUM_SCALE    # 2^-11 : ht (=h*8192, bf16) -> h*4 fp8

CDT = mybir.dt.bfloat16
F8 = mybir.dt.float8e4
NP_CDT = ml_dtypes.bfloat16
NP_F8 = ml_dtypes.float8_e4m3
DR = mybir.MatmulPerfMode.DoubleRow

P = 128
F32 = mybir.dt.float32


class _TileContextSplitWait(tile.TileContext):
    """The walrus build in this environment rejects >1 sync-wait on a
    CTRL (Drain) instruction.  Split the kernel-tail drain's waits into
    single-wait nops emitted just before it."""

    def _drain_and_barrier(self, tick_clock, wait_clock):
        probe = self.nc.sync.nop(nofuse=True)
        wait_clock.add_sem_waits(
            probe.ins, ScopedClock({None: tick_clock.global_clock})
        )
        si = probe.ins.sync_info
        if si is not None and len(si.on_wait) > 1:
            waits = list(si.on_wait)
            probe.ins.sync_info = mybir.SyncInfo(
                on_wait=waits[:1], on_update=list(si.on_update)
            )
            for w in waits[1:]:
                n = self.nc.sync.nop(nofuse=True)
                n.ins.sync_info = mybir.SyncInfo(on_wait=[w], on_update=[])
        self.nc.sync.drain()
        self.nc.all_engine_barrier()
        assert self.sems is not None
        popped = self.nc._tile_sem_poison_stack.pop()
        assert popped is self._sem_poison
        self.nc.clear_and_free_semaphores(list(self.sems.allocated().values()))
        self.nc.all_engine_barrier()


def _split_multi_waits(bir_bytes):
    """The walrus build here accepts at most one sync-wait command per
    instruction (setupSyncWait raises 'Too many sync wait commands').
    Tile attaches however many the dependence analysis needs, so move
    extra waits onto NoOp instructions inserted just before, on the same
    engine's stream - semantically identical, codegen-compatible."""
    bir = json.loads(bir_bytes)
    for fn in bir["functions"]:
        for blk in fn["blocks"]:
            insts = blk.get("instructions")
            if not insts:
                continue
            out = []
            changed = False
            for inst in insts:
                si = inst.get("sync_info")
                waits = (si or {}).get("on_wait") or []
                if len(waits) > 1:
                    changed = True
                    for j, w in enumerate(waits[:-1]):
                        out.append(
                            {
                                "debug": inst.get("debug"),
                                "engine": inst["engine"],
                                "ins": [],
                                "name": f"{inst['name']}-w{j}",
                                "opcode": "NoOp",
                                "outs": [],
                                "sync_info": {"on_update": [], "on_wait": [w]},
                            }
                        )
                    si["on_wait"] = waits[-1:]
                out.append(inst)
            if changed:
                blk["instructions"] = out
    return json.dumps(bir).encode()


def _build_nc():
    nc = bass.Bass(target_bir_lowering=False)

    # DRAM layouts (host pre-arranged, see _prep_inputs):
    #   xb  [128, NCH, 16, TC]        bf16  x, slice-major per chunk
    #   xq  [128, NCH, 2*AP8, TC]     fp8   x*16, slices 0..2*AP8
    #   w1b [64, 128, 16-2*GP, 128]   bf16  w1*8192, per h-tile unit
    #   w2b [64, 128, 16-2*UP, 128]   bf16  w2*8192
    #   w2q [64, 128, 2*UP, 128]      fp8   w2*512
    #   w3b [16, 128, 64-2*DP, 128]   bf16  w3 (plain)
    #   w3q [16, 128, 2*DP, 128]      fp8   w3*2048
    #   outt [EMB, T_SHARD]           f32
    xb = nc.dram_tensor("xb", [P, N_CHUNKS, E_SUB, T_CHUNK], CDT,
                        kind="ExternalInput")
    w1b = nc.dram_tensor("w1b", [HT_TOTAL, P, E_SUB - 2 * GP, P], CDT,
                         kind="ExternalInput")
    w2b = nc.dram_tensor("w2b", [HT_TOTAL, P, E_SUB - 2 * UP, P], CDT,
                         kind="ExternalInput")
    w3b = nc.dram_tensor("w3b", [E_SUB, P, H_SUB - 2 * DP, P], CDT,
                         kind="ExternalInput")
    if AP8:
        xq = nc.dram_tensor("xq", [P, N_CHUNKS, 2 * AP8, T_CHUNK], F8,
                            kind="ExternalInput")
    if UP:
        w2q = nc.dram_tensor("w2q", [HT_TOTAL, P, 2 * UP, P], F8,
                             kind="ExternalInput")
    if DP:
        w3q = nc.dram_tensor("w3q", [E_SUB, P, 2 * DP, P], F8,
                             kind="ExternalInput")
    # fp16 output: halves the tail DMA-out, costs ~2e-4 rel err (absmax
    # ~11 is far inside fp16 range); the host upcasts back to f32.
    outt = nc.dram_tensor("outt", [EMB, T_SHARD], mybir.dt.float16,
                          kind="ExternalOutput")

    # partition-major views of the weight tensors
    w1v = w1b[:].rearrange("t p e m -> p t e m")   # [128, 64, 16-2GP, 128]
    w2v = w2b[:].rearrange("t p e m -> p t e m")
    if UP:
        w2qv = w2q[:].rearrange("t p e m -> p t e m")

    n_up_bf = E_SUB - 2 * UP
    n_gp_bf = E_SUB - 2 * GP
    n_dn_bf = H_SUB - 2 * DP

    with _TileContextSplitWait(nc) as tc:
        with (
            tc.tile_pool(name="xp", bufs=1) as xp,
            tc.tile_pool(name="xqp", bufs=2) as xqp,
            tc.tile_pool(name="wp", bufs=3) as wp,
            tc.tile_pool(name="w3p", bufs=2) as w3p,
            tc.tile_pool(name="htp", bufs=1) as htp,
            tc.tile_pool(name="slp", bufs=3) as slp,
            tc.tile_pool(name="op", bufs=3) as op,
            tc.tile_pool(name="ps", bufs=2, space="PSUM") as ps,
        ):
            # ---- PE warmup: the tensor engine clock ramps up only after
            # ~3us of continuous activity (first real matmuls otherwise
            # run 2-3x slow).  Burn dummy matmuls on zeroed SBUF during
            # the ~14us DMA fill window, sized to finish just before the
            # first real operands land.  Uses a rotating po PSUM buffer
            # that phase B reuses long after.
            warm = slp.tile([P, 256], CDT, name="warm", bufs=1)
            nc.vector.memset(warm[:], 0.0)
            warmps = ps.tile([P, T_CHUNK], F32, name="po")
            for i in range(28):
                nc.tensor.matmul(
                    warmps[:, 0:256], warm[:, 0:P], warm[:],
                    start=True, stop=True,
                )

            for c in range(N_CHUNKS):
                # ---- x loads: fp8 slices first (tiny -> earliest PE start)
                if AP8:
                    xq8 = xqp.tile([P, 2 * AP8, T_CHUNK], F8, name="xq8")
                    nc.sync.dma_start(xq8[:], xq[:, c, :, :])
                xparts = [
                    xp.tile([P, 2, T_CHUNK], CDT, name=f"xs{q}")
                    for q in range(8)
                ]

                def xview(e, xparts=xparts):
                    return xparts[e // 2][:, e % 2, :]

                ht = htp.tile([P, H_SUB, T_CHUNK], CDT, name="ht")
                if DP:
                    ht8 = htp.tile([P, 2 * DP, T_CHUNK], F8, name="ht8")

                def emit_up(pu, ti, w2qs, w2s, w2view=None):
                    # bf16 body first (the full-width start=True zeroes
                    # the whole PSUM bank; a DR half-width start would
                    # wipe the other half), fp8 DoubleRow tail
                    # accumulates with start=False.
                    for e in range(2 * UP, E_SUB):
                        wv = (w2view(e) if w2view is not None
                              else w2s[:, ti, e - 2 * UP, :])
                        nc.tensor.matmul(
                            pu[:],
                            wv,
                            xview(e),
                            start=(e == 2 * UP),
                            stop=(UP == 0 and e == E_SUB - 1),
                        )
                    for pr in range(UP):
                        for m in (0, 256):
                            nc.tensor.matmul(
                                pu[:, m : m + 256],
                                w2qs[:, ti, 2 * pr : 2 * pr + 2, :],
                                xq8[:, 2 * pr : 2 * pr + 2, m : m + 256],
                                start=False,
                                stop=(pr == UP - 1 and m == 256),
                                perf_mode=DR,
                            )

                def emit_gate_finish(hs, ti, w1s, pu):
                    pg = ps.tile([P, T_CHUNK], F32, name="pg")
                    for e in range(2 * GP, E_SUB):
                        nc.tensor.matmul(
                            pg[:],
                            w1s[:, ti, e - 2 * GP, :],
                            xview(e),
                            start=(e == 2 * GP),
                            stop=(e == E_SUB - 1),
                        )
                    sl = slp.tile([P, T_CHUNK], CDT, name="sl")
                    nc.scalar.activation(
                        sl[:], pg[:], mybir.ActivationFunctionType.Silu,
                        scale=INV_PSUM,
                    )
                    nc.vector.tensor_mul(ht[:, hs, :], sl[:], pu[:])
                    if hs < 2 * DP:
                        nc.scalar.activation(
                            ht8[:, hs, :], ht[:, hs, :],
                            mybir.ActivationFunctionType.Copy,
                            scale=HT_TO_H8,
                        )

                # ---------------- phase A: gate/up + silu*up -> hT
                if c == 0:
                    # Startup is DMA-bandwidth-bound: run the up groups of
                    # the first four h-tiles back-to-back (they need only
                    # x + ~0.5MB of w2 each), parking the results in four
                    # PSUM banks, and defer their gate groups until the
                    # 2MB of w1 has streamed in behind.
                    NPRO = 4
                    hwu = n_up_bf // 2
                    # x part q holds e-slices (2q, 2q+1); the first up
                    # matmul (e=2) needs part 1, part 0 only at gate(0).
                    xorder = [[1, 2], [3, 4], [5, 6], [7, 0]]
                    pro_w2q, pro_w2s = [], []
                    for t in range(NPRO):
                        if UP:
                            q8 = wp.tile([P, 1, 2 * UP, P], F8,
                                         name="w2q1", bufs=2)
                            nc.sync.dma_start(q8[:], w2qv[:, t : t + 1, :, :])
                        else:
                            q8 = None
                        pro_w2q.append(q8)
                        if t == 0:
                            # first-needed weights split in two half-DMAs
                            # so the very first matmul waits on ~half
                            sa = wp.tile([P, 1, hwu, P], CDT, name="w2sha",
                                         bufs=1)
                            nc.sync.dma_start(sa[:], w2v[:, 0:1, 0:hwu, :])
                            nc.sync.dma_start(xparts[1][:], xb[:, c, 2:4, :])
                            sb = wp.tile([P, 1, n_up_bf - hwu, P], CDT,
                                         name="w2shb", bufs=1)
                            nc.sync.dma_start(sb[:], w2v[:, 0:1, hwu:, :])
                            nc.sync.dma_start(xparts[2][:], xb[:, c, 4:6, :])
                            pro_w2s.append((sa, sb))
                            continue
                        s = wp.tile([P, 1, n_up_bf, P], CDT, name="w2s1",
                                    bufs=2)
                        nc.sync.dma_start(s[:], w2v[:, t : t + 1, :, :])
                        pro_w2s.append(s)
                        for q in xorder[t]:
                            nc.sync.dma_start(
                                xparts[q][:], xb[:, c, 2 * q : 2 * q + 2, :]
                            )
                    pro_w1 = []
                    for t in range(NPRO):
                        w = wp.tile([P, 1, n_gp_bf, P], CDT, name="w1s1",
                                    bufs=2)
                        nc.sync.dma_start(w[:], w1v[:, t : t + 1, :, :])
                        pro_w1.append(w)
                    pus = []
                    for t in range(NPRO):
                        pu = ps.tile([P, T_CHUNK], F32, name="pu", bufs=4)
                        if t == 0:
                            sa, sb = pro_w2s[0]

                            def w2v0(e, sa=sa, sb=sb, hwu=hwu):
                                k = e - 2 * UP
                                return (sa[:, 0, k, :] if k < hwu
                                        else sb[:, 0, k - hwu, :])

                            emit_up(pu, 0, pro_w2q[t], None, w2view=w2v0)
                        else:
                            emit_up(pu, 0, pro_w2q[t], pro_w2s[t])
                        pus.append(pu)
                    for t in range(NPRO):
                        emit_gate_finish(t, 0, pro_w1[t], pus[t])
                    tile0 = NPRO
                else:
                    for q in range(8):
                        nc.sync.dma_start(
                            xparts[q][:], xb[:, c, 2 * q : 2 * q + 2, :]
                        )
                    tile0 = 0

                for tstart in range(tile0, HT_TOTAL, 2):
                    nt = 2
                    if UP:
                        w2qs = wp.tile([P, nt, 2 * UP, P], F8,
                                       name=f"w2q{nt}")
                        nc.sync.dma_start(
                            w2qs[:], w2qv[:, tstart : tstart + nt, :, :]
                        )
                    w2s = wp.tile([P, nt, n_up_bf, P], CDT,
                                  name=f"w2s{nt}")
                    nc.sync.dma_start(
                        w2s[:], w2v[:, tstart : tstart + nt, :, :]
                    )
                    w1s = wp.tile([P, nt, n_gp_bf, P], CDT,
                                  name=f"w1s{nt}")
                    nc.sync.dma_start(
                        w1s[:], w1v[:, tstart : tstart + nt, :, :]
                    )
                    for ti in range(nt):
                        hs = tstart + ti
                        pu = ps.tile([P, T_CHUNK], F32, name="pu", bufs=4)
                        emit_up(pu, ti, w2qs, w2s)
                        emit_gate_finish(hs, ti, w1s, pu)

                # ---------------- phase B: outT = sum_h w3T^T @ hT
                hh = n_dn_bf // 2
                t0 = c * T_CHUNK
                for et in range(E_SUB):
                    e0 = et * P
                    if DP:
                        w3qs = w3p.tile([P, 2 * DP, P], F8, name="w3q")
                        nc.sync.dma_start(w3qs[:], w3q[et])
                    # two half-slabs: accumulation can start when the
                    # first ~1MB lands instead of waiting for all 2MB
                    w3a = w3p.tile([P, hh, P], CDT, name="w3a")
                    nc.sync.dma_start(w3a[:], w3b[et, :, 0:hh, :])
                    w3c = w3p.tile([P, n_dn_bf - hh, P], CDT, name="w3c")
                    nc.sync.dma_start(w3c[:], w3b[et, :, hh:, :])
                    po = ps.tile([P, T_CHUNK], F32, name="po")
                    for h in range(2 * DP, H_SUB):
                        hb = h - 2 * DP
                        w3vv = w3a[:, hb, :] if hb < hh else w3c[:, hb - hh, :]
                        nc.tensor.matmul(
                            po[:],
                            w3vv,
                            ht[:, h, :],
                            start=(h == 2 * DP),
                            stop=(DP == 0 and h == H_SUB - 1),
                        )
                    for pr in range(DP):
                        for m in (0, 256):
                            nc.tensor.matmul(
                                po[:, m : m + 256],
                                w3qs[:, 2 * pr : 2 * pr + 2, :],
                                ht8[:, 2 * pr : 2 * pr + 2, m : m + 256],
                                start=False,
                                stop=(pr == DP - 1 and m == 256),
                                perf_mode=DR,
                            )
                    ot = op.tile([P, T_CHUNK], mybir.dt.float16, name="ot")
                    nc.vector.tensor_scalar_mul(ot[:], po[:], INV_PSUM)
                    nc.sync.dma_start(
                        outt[e0 : e0 + P, t0 : t0 + T_CHUNK], ot[:]
                    )

    fixed = _split_multi_waits(bass.Bass.to_json_bytes(nc))
    nc.to_json_bytes = lambda: fixed
    return nc


_nc_cache = None


def _get_nc():
    global _nc_cache
    if _nc_cache is None:
        _nc_cache = _build_nc()
    return _nc_cache


def _prep_inputs(x, w1, w2, w3):
    X = x.reshape(T_TOTAL, EMB)

    # weights are shared across cores
    # w1b[t,p,e,m] = 8192*w1[t*128+m, (e+2*GP)*128+p]
    w1r = w1.reshape(HT_TOTAL, P, E_SUB, P)          # [t, m, e, p]
    w1b = np.ascontiguousarray(
        (w1r[:, :, 2 * GP :, :] * PSUM_SCALE).transpose(0, 3, 2, 1)
    ).astype(NP_CDT)
    w2r = w2.reshape(HT_TOTAL, P, E_SUB, P)
    w2b = np.ascontiguousarray(
        (w2r[:, :, 2 * UP :, :] * PSUM_SCALE).transpose(0, 3, 2, 1)
    ).astype(NP_CDT)
    # w3b[et,p,hs,m] = w3[et*128+m, (hs+2*DP)*128+p]
    w3r = w3.reshape(E_SUB, P, H_SUB, P)             # [et, m, hs, p]
    w3b = np.ascontiguousarray(
        w3r[:, :, 2 * DP :, :].transpose(0, 3, 2, 1)
    ).astype(NP_CDT)

    shared = {"w1b": w1b, "w2b": w2b, "w3b": w3b}
    if UP:
        shared["w2q"] = np.ascontiguousarray(
            (w2r[:, :, : 2 * UP, :] * SW12).transpose(0, 3, 2, 1)
        ).astype(NP_F8)
    if DP:
        shared["w3q"] = np.ascontiguousarray(
            (w3r[:, :, : 2 * DP, :] * SW3).transpose(0, 3, 2, 1)
        ).astype(NP_F8)

    in_maps = []
    for i in range(N_CORES):
        Xi = X[i * T_SHARD : (i + 1) * T_SHARD]      # [T_SHARD, EMB]
        # xb[p,c,e,t] = Xi[c*TC+t, e*128+p]
        xr = Xi.reshape(N_CHUNKS, T_CHUNK, E_SUB, P)  # [c, t, e, p]
        m = {
            "xb": np.ascontiguousarray(
                xr.transpose(3, 0, 2, 1)
            ).astype(NP_CDT)
        }
        if AP8:
            m["xq"] = np.ascontiguousarray(
                (xr[:, :, : 2 * AP8, :] * SX8).transpose(3, 0, 2, 1)
            ).astype(NP_F8)
        m.update(shared)
        in_maps.append(m)
    return in_maps


def kernel(x, w1, w2, w3, scale_x=None, _trace=False):
    x = np.asarray(x, np.float32)
    w1 = np.asarray(w1, np.float32)
    w2 = np.asarray(w2, np.float32)
    w3 = np.asarray(w3, np.float32)

    nc = _get_nc()
    in_maps = _prep_inputs(x, w1, w2, w3)
    res = run_bass_kernel_spmd(nc, in_maps, list(range(N_CORES)), trace=_trace)

    outt = np.concatenate(
        [np.asarray(res.results[i]["outt"]) for i in range(N_CORES)], axis=1
    )  # [E, T_total]
    out = np.ascontiguousarray(outt.T).reshape(4, 2048, EMB).astype(np.float32)
    if _trace:
        kernel.last_results = res
    return out


if __name__ == "__main__":
    rng = np.random.default_rng(0)
    x = rng.standard_normal((4, 2048, EMB), dtype=np.float32)
    w1 = (rng.standard_normal((HID, EMB), dtype=np.float32) * 0.03).astype(
        np.float32
    )
    w2 = (rng.standard_normal((HID, EMB), dtype=np.float32) * 0.03).astype(
        np.float32
    )
    w3 = (rng.standard_normal((EMB, HID), dtype=np.float32) * 0.015).astype(
        np.float32
    )
    out = kernel(x, w1, w2, w3)
    print("out", out.shape, out.dtype, float(np.abs(out).mean()))
